# revision 12
# baseline (speedup 1.0000x reference)
"""Trainium2 Bass kernel for sparse-attention block (LSH-pooled attention + MLP).

Self-contained: accepts FULL inputs, shards batch across 8 NeuronCores,
returns FULL output. All shapes hardcoded for:
  x [16, 8192, 256], rotations [1, 256, 4, 4], q_w [256,256], kv_w [256,512],
  fc1_w [256,1024], fc2_w [1024,256], norm/bias vectors [256]/[1024].

v2 design notes:
 - Three passes per batch: A (LN1+hash+pool), B1 (attention), B2 (MLP), so the
   scalar engine stays within one activation-table set per pass
   (natural_log_exp for A/B1, gelu for B2).
 - rstd = exp(-0.5*ln(var+eps)) keeps LN math in the exp table set.
 - Rotation projection uses XT tiles as the matmul stationary operand, giving
   bucket scores directly in natural (token-major) layout.
 - fc2 runs in fp8 DoubleRow (gelu writes fp8 hc for free); fc1 stays bf16.
 - x is converted to bf16 on the host; the output DRAM tensor is bf16 and is
   upcast to f32 on the host. x2 (attention residual) overwrites the X_ trunk.
"""

import sys

sys.path.insert(0, "/opt/trn_rl_repo")

from contextlib import ExitStack

import ml_dtypes
import numpy as np

import concourse.bass as bass
import concourse.tile as tile
from concourse import bacc, mybir
from concourse.bass_utils import run_bass_kernel_spmd
from concourse.masks import make_identity

F32 = mybir.dt.float32
BF16 = mybir.dt.bfloat16
FP8 = mybir.dt.float8e4

N_CORES = 8
B, N, C = 16, 8192, 256
BPC = B // N_CORES          # batches per core
H, DH = 8, 32               # heads
NH, NB = 4, 8               # hashes, buckets
M = NH * NB                 # 32 pooled tokens
DFF = 4 * C                 # 1024
P = 128
TT = N // P                 # 64 token tiles per batch
CH = 512                    # chunk = 4 token tiles
NCHUNK = N // CH            # 16
TPC = CH // P               # 4 tiles per chunk
LN_EPS = 1e-5
AF = mybir.ActivationFunctionType
ALU = mybir.AluOpType


def _pass_a(nc, xr, W, T, pools):
    """LN1 stats+apply, transpose, rotation hash, pooling accumulate."""
    sb_chunk, sb_cbig, ps_t, ps_mm2, ps_sm = pools
    X_, XT, MV, RSD, XA = T["X_"], T["XT"], T["MV"], T["RSD"], T["XA"]
    IDENT = W["IDENT"]

    ps_pool = ps_sm.tile([M, 512], F32, tag="acc")

    # stage 1: load all chunks, LN1 stats; one batched Ln+Exp for rstd
    for c in range(NCHUNK):
        csl = slice(c * TPC, (c + 1) * TPC)
        nc.sync.dma_start(XA[:, csl, :], xr[:, csl, :])
        st = sb_chunk.tile([P, TPC, 6], F32, tag="bnst")
        for i in range(TPC):
            nc.vector.bn_stats(out=st[:, i], in_=XA[:, c * TPC + i, :])
            nc.vector.bn_aggr(out=MV[:, c * TPC + i, :], in_=st[:, i])
    lnv = sb_chunk.tile([P, TT], F32, tag="lnv")
    nc.scalar.activation(lnv[:], MV[:, :, 1], AF.Ln, bias=W["EPS"][:])
    nc.scalar.activation(RSD[:], lnv[:], AF.Exp, scale=-0.5)
    nc.vector.tensor_tensor(T["MRN"][:], MV[:, :, 0], RSD[:], ALU.mult)
    nc.gpsimd.tensor_scalar_mul(T["MRN"][:], T["MRN"][:], -1.0)

    # stage 2: normalize, transpose, hash, pool
    for c in range(NCHUNK):
        for i in range(TPC):
            t = c * TPC + i
            xa = XA[:, c * TPC : (c + 1) * TPC, :]
            # normalize -> X_ (bf16); alternate DVE/ACT to balance load
            if t % 2 == 0:
                nc.vector.tensor_scalar(
                    out=X_[:, t, 0:C],
                    in0=xa[:, i, :],
                    scalar1=MV[:, t, 0:1],
                    scalar2=RSD[:, t : t + 1],
                    op0=ALU.subtract,
                    op1=ALU.mult,
                )
            else:
                nc.scalar.activation(
                    X_[:, t, 0:C], xa[:, i, :], AF.Identity,
                    bias=T["MRN"][:, t : t + 1], scale=RSD[:, t : t + 1],
                )
            # transpose both halves into one PSUM tile, single copy out
            pst = ps_t.tile([P, 2, P], BF16, tag="pst")
            for h in range(2):
                nc.tensor.transpose(pst[:, h, :], X_[:, t, h * P : (h + 1) * P], IDENT[:])
            nc.vector.tensor_copy(XT[:, :, t, :], pst[:])
            # rotation scores, natural layout: psr[tok, 16] = XT_tile.T @ ROT
            psr_t = ps_t.tile([P, 2, P], F32, tag="pst")
            psr = psr_t[:, 0, 0:16]
            nc.tensor.matmul(psr, XT[:, 0, t, :], W["ROT"][:, 0, :], start=True, stop=False)
            nc.tensor.matmul(psr, XT[:, 1, t, :], W["ROT"][:, 1, :], start=False, stop=True)
            # bucket one-hot: |r| max per hash, compare +/-
            rt = sb_chunk.tile([P, NH, NH], F32, tag="rt")
            nc.vector.tensor_copy(rt[:], psr.rearrange("p (h i) -> p h i", h=NH))
            am = sb_chunk.tile([P, NH], F32, tag="am")
            nc.vector.tensor_reduce(
                out=am[:], in_=rt[:], axis=mybir.AxisListType.X,
                op=ALU.max, apply_absolute_value=True,
            )
            nam = sb_chunk.tile([P, NH], F32, tag="nam")
            nc.gpsimd.tensor_scalar_mul(nam[:], am[:], -1.0)
            oh = sb_chunk.tile([P, NH, NB], BF16, tag="oh")
            nc.vector.tensor_tensor(
                oh[:, :, 0:NH], rt[:], am[:, :, None].to_broadcast((P, NH, NH)),
                ALU.is_equal,
            )
            nc.vector.tensor_tensor(
                oh[:, :, NH:NB], rt[:], nam[:, :, None].to_broadcast((P, NH, NH)),
                ALU.is_equal,
            )
            # pooling accumulate: [32 x 257] += one_hot^T @ [x_ | 1]
            nc.tensor.matmul(
                ps_pool[:, 0 : C + 1],
                oh[:].rearrange("p h b -> p (h b)"),
                X_[:, t, 0 : C + 1],
                start=(t == 0), stop=(t == TT - 1), skip_group_check=True,
            )
    return ps_pool


def _kv_section(nc, W, T, pools, ps_pool):
    """pooled sums -> k-hat / v-hat block-diagonal tiles."""
    sb_chunk, sb_cbig, ps_t, ps_mm2, ps_sm = pools
    IDENT = W["IDENT"]
    sb = sb_chunk

    pcb = sb.tile([M, C], BF16, tag="pcb")
    nc.vector.tensor_copy(pcb[:], ps_pool[:, 0:C])
    invc = sb.tile([M, 1], F32, tag="invc")
    nc.vector.tensor_scalar_add(invc[:], ps_pool[:, C : C + 1], 1e-20)
    nc.vector.reciprocal(invc[:], invc[:])
    ptb = sb.tile([P, 2, M], BF16, tag="ptb")
    pstp = ps_t.tile([P, 2, P], BF16, tag="pst")
    for h in range(2):
        nc.tensor.transpose(pstp[:, h, 0:M], pcb[:, h * P : (h + 1) * P], IDENT[:M, :M])
    nc.vector.tensor_copy(ptb[:], pstp[:, :, 0:M])
    # kv = pooled^T.T @ kv_w, then scale rows by 1/count
    pskv = ps_mm2.tile([M, 2 * C], F32, tag="mm2")
    nc.tensor.matmul(pskv[:], ptb[:, 0, :], W["WKV"][:, 0, :], start=True, stop=False)
    nc.tensor.matmul(pskv[:], ptb[:, 1, :], W["WKV"][:, 1, :], start=False, stop=True)
    kv = sb.tile([M, 2 * C], BF16, tag="kv")
    nc.vector.tensor_scalar_mul(kv[:], pskv[:], invc[:])
    khat = sb.tile([P, 2, P], BF16, tag="khat")
    vhat = sb.tile([P, 2, P], BF16, tag="vhat")
    nc.vector.memset(khat[:], 0.0)
    nc.vector.memset(vhat[:], 0.0)
    for h2 in range(2):
        pskt_t = ps_t.tile([P, 2, P], BF16, tag="pst")
        pskt = pskt_t[:, 0]
        nc.tensor.transpose(pskt[:, 0:M], kv[:, h2 * P : (h2 + 1) * P], IDENT[:M, :M])
        for j in range(4):
            nc.vector.tensor_copy(
                khat[32 * j : 32 * (j + 1), h2, 32 * j : 32 * (j + 1)],
                pskt[32 * j : 32 * (j + 1)][:, 0:M],
            )
            nc.gpsimd.tensor_copy(
                vhat[32 * j : 32 * (j + 1), h2, 32 * j : 32 * (j + 1)],
                kv[:, C + h2 * P + 32 * j : C + h2 * P + 32 * (j + 1)],
            )
    # effective keys: KET[m', c] = sum_d k[m', d] * wq_scaled[c, 32h+d]
    psket = ps_mm2.tile([P, 2, C], F32, tag="mm2")
    for h2 in range(2):
        nc.tensor.matmul(
            psket[:, h2, :], khat[:, h2, :], W["WQT"][:, h2, :],
            start=True, stop=True, skip_group_check=True,
        )
    kes = sb.tile([P, 2, C], BF16, tag="kes")
    nc.vector.tensor_copy(kes[:], psket[:])
    KEH = sb.tile([P, 2, 2, P], BF16, tag="keh")
    for k2 in range(2):
        psket_t = ps_t.tile([P, 2, P], BF16, tag="pst")
        for h2 in range(2):
            nc.tensor.transpose(
                psket_t[:, h2, :], kes[:, h2, k2 * P : (k2 + 1) * P], IDENT[:]
            )
        nc.vector.tensor_copy(KEH[:, k2, :, :], psket_t[:])
    return KEH, vhat


def _pass_b1(nc, xr, W, T, pools, KEH, vhat):
    """Attention: scores via effective keys, softmax, AV, residual into X2."""
    sb_chunk, sb_cbig, ps_t, ps_mm2, ps_sm = pools
    X2, XT, MV2, RSD2, ZN = T["X2"], T["XT"], T["MV2"], T["RSD2"], T["ZN"]
    IDENT = W["IDENT"]

    XA = T["XA"]
    for c in range(NCHUNK):
        csl = slice(c * TPC, (c + 1) * TPC)
        xb2 = XA[:, csl, :]
        # scores directly from x^T via effective keys: K=256 contraction
        psa = ps_mm2.tile([P, 2, CH], F32, tag="mm2")
        for h2 in range(2):
            nc.tensor.matmul(
                psa[:, h2, :], KEH[:, 0, h2, :], XT[:, 0, csl, :],
                start=True, stop=False, skip_group_check=True,
            )
            nc.tensor.matmul(
                psa[:, h2, :], KEH[:, 1, h2, :], XT[:, 1, csl, :],
                start=False, stop=True, skip_group_check=True,
            )
        expc = sb_chunk.tile([P, 2, CH], BF16, tag="expc")
        nc.scalar.activation(expc[:], psa[:], AF.Exp)
        # Z per (head, token), feature-major [8, CH]
        psz = ps_sm.tile([H, CH], F32, tag="acc")
        for h2 in range(2):
            nc.tensor.matmul(
                psz[:], W["SB8"][:, h2, :], expc[:, h2, :],
                start=(h2 == 0), stop=(h2 == 1), skip_group_check=True,
            )
        zsb = sb_chunk.tile([H, CH], BF16, tag="zsb")
        nc.vector.tensor_copy(zsb[:], psz[:])
        # Z -> natural [tok, 8] via 4 transposes into one psum tile; reciprocal
        psznat_t = ps_t.tile([P, 2, P], BF16, tag="pst")
        psznat = psznat_t[:].rearrange("p a b -> p (a b)")[:, 0 : TPC * H].rearrange(
            "p (i h) -> p i h", h=H
        )
        for i in range(TPC):
            nc.tensor.transpose(psznat[:, i, :], zsb[:, i * P : (i + 1) * P], IDENT[:H, :H])
        nc.vector.reciprocal(out=ZN[:, csl, :], in_=psznat)
        # AV (unnormalized), feature-major
        pso = ps_mm2.tile([P, 2, CH], F32, tag="mm2")
        for h2 in range(2):
            nc.tensor.matmul(
                pso[:, h2, :], vhat[:, h2, :], expc[:, h2, :],
                start=True, stop=True, skip_group_check=True,
            )
        ot = sb_chunk.tile([P, 2, CH], BF16, tag="ot")
        nc.scalar.activation(ot[:], pso[:], AF.Copy)
        # per tile: transpose out, scale by 1/Z, add residual -> X_ (now x2)
        for i in range(TPC):
            t = c * TPC + i
            psn = ps_t.tile([P, 2, P], BF16, tag="pst")
            for h2 in range(2):
                nc.tensor.transpose(psn[:, h2, :], ot[:, h2, i * P : (i + 1) * P], IDENT[:])
            tmp = sb_chunk.tile([P, H, DH], BF16, tag="tmp")
            nc.vector.tensor_tensor(
                tmp[:],
                psn[:].rearrange("p a b -> p (a b)").rearrange("p (h d) -> p h d", h=H),
                ZN[:, t, :, None].to_broadcast((P, H, DH)),
                ALU.mult,
            )
            nc.vector.tensor_tensor(
                X2[:, t, :], tmp[:].rearrange("p h d -> p (h d)"), xb2[:, i, :],
                ALU.add,
            )
        # LN2 stats on x2
        st2 = sb_chunk.tile([P, TPC, 6], F32, tag="bnst2")
        for i in range(TPC):
            t = c * TPC + i
            nc.vector.bn_stats(out=st2[:, i], in_=X2[:, t, :])
            nc.vector.bn_aggr(out=MV2[:, t, :], in_=st2[:, i])
    lnv2 = sb_chunk.tile([P, TT], F32, tag="lnv2")
    nc.scalar.activation(lnv2[:], MV2[:, :, 1], AF.Ln, bias=W["EPS"][:])
    nc.scalar.activation(RSD2[:], lnv2[:], AF.Exp, scale=-0.5)
    nc.vector.tensor_tensor(T["MRN2"][:], MV2[:, :, 0], RSD2[:], ALU.mult)
    nc.gpsimd.tensor_scalar_mul(T["MRN2"][:], T["MRN2"][:], -1.0)


def _pass_b2(nc, orr, W, T, pools):
    """MLP (gelu table): LN2 apply, y^T, fc1+gelu(fp8 out), fc2 fp8 DR, out."""
    sb_chunk, sb_cbig, ps_t, ps_mm2, ps_sm = pools
    X2, MV2, RSD2 = T["X2"], T["MV2"], T["RSD2"]
    IDENT = W["IDENT"]

    for c in range(NCHUNK):
        csl = slice(c * TPC, (c + 1) * TPC)
        # LN2 apply + transpose to y^T
        yt = sb_chunk.tile([P, 2, TPC, P], BF16, tag="yt")
        for i in range(TPC):
            t = c * TPC + i
            yc = sb_chunk.tile([P, C], BF16, tag="yc")
            if t % 2 == 0:
                nc.vector.tensor_scalar(
                    out=yc[:], in0=X2[:, t, :],
                    scalar1=MV2[:, t, 0:1], scalar2=RSD2[:, t : t + 1],
                    op0=ALU.subtract, op1=ALU.mult,
                )
            else:
                nc.scalar.activation(
                    yc[:], X2[:, t, :], AF.Identity,
                    bias=T["MRN2"][:, t : t + 1], scale=RSD2[:, t : t + 1],
                )
            psy_t = ps_t.tile([P, 2, P], BF16, tag="pst")
            for h in range(2):
                nc.tensor.transpose(psy_t[:, h, :], yc[:, h * P : (h + 1) * P], IDENT[:])
            nc.vector.tensor_copy(yt[:, :, i, :], psy_t[:])
        ytf = yt[:].rearrange("p k i q -> p k (i q)")
        # fc1 (bf16) in 2-mblock pairs -> gelu (bias fused) -> hc (fp8)
        hc = T["sb_hc"].tile([P, 8, CH], BF16, tag="hc")
        for mp in range(4):
            psh = ps_mm2.tile([P, 2, CH], F32, tag="mm2")
            for mi in range(2):
                m = 2 * mp + mi
                nc.tensor.matmul(
                    psh[:, mi, :], W["W1"][:, 0, m * P : (m + 1) * P], ytf[:, 0, :],
                    start=True, stop=False, skip_group_check=True,
                )
                nc.tensor.matmul(
                    psh[:, mi, :], W["W1"][:, 1, m * P : (m + 1) * P], ytf[:, 1, :],
                    start=False, stop=True, skip_group_check=True,
                )
                nc.scalar.activation(
                    hc[:, m, :], psh[:, mi, :], AF.Gelu, bias=W["B1T"][:, m : m + 1],
                )
        # fc2 (bf16): K=1024 in 8 k-blocks, both m2 into one psum tile
        psy = ps_mm2.tile([P, 2, CH], F32, tag="mm2")
        for m2 in range(2):
            for k in range(8):
                nc.tensor.matmul(
                    psy[:, m2, :],
                    W["W2"][:, k, m2 * P : (m2 + 1) * P],
                    hc[:, k, :],
                    start=(k == 0), stop=(k == 7),
                    skip_group_check=True,
                )
        yo = sb_chunk.tile([P, 2, CH], BF16, tag="yo")
        for m2 in range(2):
            nc.scalar.activation(
                yo[:, m2, :], psy[:, m2, :], AF.Identity, bias=W["B2T"][:, m2 : m2 + 1]
            )
        # transpose back + residual + store
        outc = sb_cbig.tile([P, TPC, C], BF16, tag="outc")
        for i in range(TPC):
            t = c * TPC + i
            psm = ps_t.tile([P, 2, P], BF16, tag="pst")
            for h2 in range(2):
                nc.tensor.transpose(psm[:, h2, :], yo[:, h2, i * P : (i + 1) * P], IDENT[:])
            nc.vector.tensor_tensor(
                outc[:, i, :], psm[:].rearrange("p a b -> p (a b)"), X2[:, t, :],
                ALU.add,
            )
        nc.sync.dma_start(orr[:, csl, :], outc[:])


def _build(affine_flags, repeat=1):
    assert not any(affine_flags), "affine path not implemented in v2"
    nc = bacc.Bacc("TRN2", target_bir_lowering=False, debug=False, enable_asserts=True)

    x_ap = nc.dram_tensor("x", [BPC, N, C], BF16, kind="ExternalInput").ap()
    wqt = nc.dram_tensor("wqt", [C, C], BF16, kind="ExternalInput").ap()
    wkv = nc.dram_tensor("wkv", [C, 2 * C], BF16, kind="ExternalInput").ap()
    rot = nc.dram_tensor("rot", [C, 16], BF16, kind="ExternalInput").ap()
    w1 = nc.dram_tensor("w1", [C, DFF], BF16, kind="ExternalInput").ap()
    w2 = nc.dram_tensor("w2", [DFF, C], BF16, kind="ExternalInput").ap()
    b1t = nc.dram_tensor("b1t", [P, 8], F32, kind="ExternalInput").ap()
    b2t = nc.dram_tensor("b2t", [P, 2], F32, kind="ExternalInput").ap()
    o_ap = nc.dram_tensor("out", [BPC, N, C], BF16, kind="ExternalOutput").ap()

    with tile.TileContext(nc) as tc:
        with ExitStack() as ctx:
            sb_w = ctx.enter_context(tc.tile_pool(name="weights", bufs=1))
            sb_trunk = ctx.enter_context(tc.tile_pool(name="trunk", bufs=1))
            sb_chunk = ctx.enter_context(tc.tile_pool(name="chunk", bufs=3))
            sb_cbig = ctx.enter_context(tc.tile_pool(name="cbig", bufs=2))
            sb_hc = ctx.enter_context(tc.tile_pool(name="hc", bufs=1))
            ps_t = ctx.enter_context(tc.tile_pool(name="ps_t", bufs=2, space="PSUM"))
            ps_mm2 = ctx.enter_context(tc.tile_pool(name="ps_mm2", bufs=2, space="PSUM"))
            ps_sm = ctx.enter_context(tc.tile_pool(name="ps_sm", bufs=1, space="PSUM"))

            W = {}
            W["IDENT"] = sb_w.tile([P, P], BF16, name="IDENT")
            make_identity(nc, W["IDENT"][:])
            W["WQT"] = sb_w.tile([P, 2, C], BF16, name="WQT")
            nc.sync.dma_start(W["WQT"][:], wqt.rearrange("(h p) c -> p h c", p=P))
            W["WKV"] = sb_w.tile([P, 2, 2 * C], BF16, name="WKV")
            nc.sync.dma_start(W["WKV"][:], wkv.rearrange("(k p) m -> p k m", p=P))
            W["ROT"] = sb_w.tile([P, 2, 16], BF16, name="ROTW")
            nc.sync.dma_start(W["ROT"][:], rot.rearrange("(k p) m -> p k m", p=P))
            W["W1"] = sb_w.tile([P, 2, DFF], BF16, name="W1")
            nc.sync.dma_start(W["W1"][:], w1.rearrange("(k p) m -> p k m", p=P))
            W["W2"] = sb_w.tile([P, 8, C], BF16, name="W2")
            nc.sync.dma_start(W["W2"][:], w2.rearrange("(k p) m -> p k m", p=P))
            W["B1T"] = sb_w.tile([P, 8], F32, name="B1T")
            nc.sync.dma_start(W["B1T"][:], b1t)
            W["B2T"] = sb_w.tile([P, 2], F32, name="B2T")
            nc.sync.dma_start(W["B2T"][:], b2t)
            W["EPS"] = sb_w.tile([P, 1], F32, name="EPS")
            nc.vector.memset(W["EPS"][:], LN_EPS)
            W["SB8"] = sb_w.tile([P, 2, 8], BF16, name="SB8")
            nc.vector.memset(W["SB8"][:], 0.0)
            for h2 in range(2):
                for jl in range(4):
                    nc.vector.memset(
                        W["SB8"][32 * jl : 32 * (jl + 1), h2, h2 * 4 + jl : h2 * 4 + jl + 1], 1.0
                    )

            T = {}
            T["sb_hc"] = sb_hc
            T["XA"] = sb_trunk.tile([P, TT, C], BF16, name="XA")
            T["X_"] = sb_trunk.tile([P, TT, C + 2], BF16, name="Xn")
            T["X2"] = sb_trunk.tile([P, TT, C], BF16, name="X2")
            T["XT"] = sb_trunk.tile([P, 2, TT, P], BF16, name="XT")
            T["MV"] = sb_trunk.tile([P, TT, 2], F32, name="MV")
            T["RSD"] = sb_trunk.tile([P, TT], F32, name="RSD")
            T["MV2"] = sb_trunk.tile([P, TT, 2], F32, name="MV2")
            T["RSD2"] = sb_trunk.tile([P, TT], F32, name="RSD2")
            T["ZN"] = sb_trunk.tile([P, TT, H], F32, name="ZN")
            T["MRN"] = sb_trunk.tile([P, TT], F32, name="MRN")
            T["MRN2"] = sb_trunk.tile([P, TT], F32, name="MRN2")
            nc.vector.memset(T["X_"][:, :, C : C + 1], 1.0)

            pools = (sb_chunk, sb_cbig, ps_t, ps_mm2, ps_sm)
            for _r in range(repeat):
                for b in range(BPC):
                    xr = x_ap[b].rearrange("(t p) c -> p t c", p=P)
                    orr = o_ap[b].rearrange("(t p) c -> p t c", p=P)
                    ps_pool = _pass_a(nc, xr, W, T, pools)
                    KEH, vhat = _kv_section(nc, W, T, pools, ps_pool)
                    _pass_b1(nc, xr, W, T, pools, KEH, vhat)
                    _pass_b2(nc, orr, W, T, pools)

    nc.compile()
    return nc


_NC_CACHE = {}


def _get_nc(affine_flags, repeat=1):
    key = (affine_flags, repeat)
    if key not in _NC_CACHE:
        _NC_CACHE[key] = _build(affine_flags, repeat)
    return _NC_CACHE[key]


def make_in_maps(x, rotations, q_w, kv_w, fc1_w, fc2_w, fc1_b, fc2_b):
    bf = ml_dtypes.bfloat16
    scale = DH ** -0.5
    common = {
        "wqt": np.ascontiguousarray((np.asarray(q_w, np.float32) * scale).T).astype(bf),
        "wkv": np.asarray(kv_w, np.float32).astype(bf),
        "rot": np.asarray(rotations, np.float32).reshape(C, NH * (NB // 2)).astype(bf),
        "w1": np.asarray(fc1_w, np.float32).astype(bf),
        "w2": np.asarray(fc2_w, np.float32).astype(bf),
        "b1t": np.ascontiguousarray(np.asarray(fc1_b, np.float32).reshape(8, P).T),
        "b2t": np.ascontiguousarray(np.asarray(fc2_b, np.float32).reshape(2, P).T),
    }
    xs = np.asarray(x, np.float32).astype(bf).reshape(N_CORES, BPC, N, C)
    return [{**common, "x": np.ascontiguousarray(xs[i])} for i in range(N_CORES)]


def kernel(
    x, rotations, norm1_g, norm1_b, q_w, kv_w, norm2_g, norm2_b,
    fc1_w, fc1_b, fc2_w, fc2_b,
):
    use_g1 = not np.allclose(np.asarray(norm1_g), 1.0)
    use_b1 = not np.allclose(np.asarray(norm1_b), 0.0)
    use_g2 = not np.allclose(np.asarray(norm2_g), 1.0)
    use_b2 = not np.allclose(np.asarray(norm2_b), 0.0)
    flags = (use_g1, use_b1, use_g2, use_b2)
    nc = _get_nc(flags)

    in_maps = make_in_maps(x, rotations, q_w, kv_w, fc1_w, fc2_w, fc1_b, fc2_b)
    res = run_bass_kernel_spmd(nc, in_maps, core_ids=list(range(N_CORES)))
    out = np.concatenate(
        [res.results[i]["out"].astype(np.float32) for i in range(N_CORES)], axis=0
    )
    return out.reshape(B, N, C)
